# revision 36
# baseline (speedup 1.0000x reference)
"""Trainium2 Bass kernel for the ADMM unrolled network (nn_ADMM).

Per batch element b (4096 total, data-parallel over 8 NeuronCores):
    A_b = H_b^T H_b + tau I   (80x80, SPD, tau constant across iterations)
    10 ADMM iterations, each requiring x = A_b^{-1} (HTy + u + tau z)
    plus elementwise tanh updates; final loss = sum |x-xt|^2 + |z-xt|^2.

Algorithm on device (per element):
  Phase A (TensorE-heavy):
    - split-bf16 exact HTH+HTy (Hhi/Hlo bf16 pair, 3 matmuls, fp32 PSUM)
    - Q = Chebyshev poly p_d(A)/2 approx of inv(A)/2, Horner in the
      normalized variable T = alpha A + beta I, bf16 intermediates
    - Qs = Q + Q^T (PE transpose; exactly symmetric)
    - one non-commutative Newton-Schulz step vs split-A:
        P1 = Qs + Qs (I - A Qs)   (valid: residual squares exactly)
      P1 stored fp32.  Applied operator is P1^T; ||I - P1^T A|| ~ 4e-5.
      P1 split into an exact bf16 pair (P1hi + P1lo), FWL-padded to 128
      weight columns for fast weight loads.
  Phase B: 10 iterations; per element x = P1^T w via two N=2 bf16 matmuls
    (stationary P1hi / P1lo, moving [w_hi | w_lo] split columns); the P1lo
    term is refreshed on iterations {0,2,4,6,8,9} and reused stale on the
    others.  Elementwise phase runs feature-major [80, 128] in two
    independently pipelined blocks on DVE/ACT/GPSIMD (tanh via ScalarE LUT).
  Measured: HW exec ~1.06 ms (fast clock mode) / ~1.27 ms (P0 downclock),
    rel_err(x) ~2.4e-3, rel_err(loss) ~1.1e-5.

Everything (shapes, tau, gammas, Chebyshev coefficients) is resolved at
kernel() call time from the actual inputs and baked into the graph.
"""

import numpy as np

B, M, N, L = 4096, 100, 80, 10
NCORES = 8
BS = B // NCORES          # 512 elements per core
HALF = 256                # phase-B batch (2 per core)
CHUNK = 16                # H DMA chunk
GRP = 4                   # phase-A group (psum batching)
DEG = 3                   # Chebyshev degree
DBG = False               # debug output dumps (small sizes only)
EV_LO, EV_HI = 0.0, 410.0  # eigenvalue range of HTH (empirical + margin)

_cache = {}


def _cheb_mono_t(deg, a, b):
    """Monomial coeffs (in t) of near-minimax poly approx of 1/x on [a,b],
    x = ((t+1)/2)*(b-a) + a."""
    kk = np.arange(deg + 1)
    xs = 0.5 * (b - a) * np.cos((2 * kk + 1) * np.pi / (2 * (deg + 1))) + 0.5 * (a + b)
    cc = np.polynomial.chebyshev.chebfit(2 * (xs - a) / (b - a) - 1, 1.0 / xs, deg)
    return np.polynomial.chebyshev.cheb2poly(cc)


def _build_graph(tau0, ilbar, taus, g1, g3):
    import concourse.bass as bass
    import concourse.mybir as mybir
    import concourse.tile as tile
    from concourse import bacc
    from concourse.alu_op_type import AluOpType

    f32 = mybir.dt.float32
    bf16 = mybir.dt.bfloat16
    AF = mybir.ActivationFunctionType

    a_, b_ = tau0 + EV_LO, tau0 + EV_HI
    alpha = 2.0 / (b_ - a_)
    beta = -2.0 * a_ / (b_ - a_) - 1.0
    ct = (_cheb_mono_t(DEG, a_, b_) * 0.5).astype(np.float32)  # halved (symmetrize)

    nc = bacc.Bacc("TRN2")
    Hd = nc.declare_dram_parameter("H", [BS, M, N], f32, isOutput=False)
    yTd = nc.declare_dram_parameter("yT", [M, BS], f32, isOutput=False)
    xiTd = nc.declare_dram_parameter("xiT", [N, BS], f32, isOutput=False)
    xtTd = nc.declare_dram_parameter("xtT", [N, BS], f32, isOutput=False)
    # consts layout (fp32, [N, 5*4*N]): tauI4 | I4 | betaI4 | m1I4 | ckI4[k=DEG-2..0]
    NCONST = 4 + (DEG - 1)
    constd = nc.declare_dram_parameter("consts", [N, NCONST * GRP * N], f32, isOutput=False)
    xoutd = nc.declare_dram_parameter("xout", [N, BS], f32, isOutput=True)
    loutd = nc.declare_dram_parameter("lout", [N, 8], f32, isOutput=True)
    if DBG:
        dbgP1h = nc.declare_dram_parameter("dbgP1h", [N, HALF * N], bf16, isOutput=True)
        dbgP1l = nc.declare_dram_parameter("dbgP1l", [N, HALF * N], bf16, isOutput=True)
        dbgHTy = nc.declare_dram_parameter("dbgHTy", [N, HALF], f32, isOutput=True)
        dbgW = nc.declare_dram_parameter("dbgW", [N, HALF], f32, isOutput=True)
        dbgX0 = nc.declare_dram_parameter("dbgX0", [N, HALF], f32, isOutput=True)
        dbgAf = nc.declare_dram_parameter("dbgAf", [N, GRP * N], f32, isOutput=True)
        dbgTb = nc.declare_dram_parameter("dbgTb", [N, GRP * N], bf16, isOutput=True)
        dbgQ = nc.declare_dram_parameter("dbgQ", [N, GRP * N], bf16, isOutput=True)
        dbgQs = nc.declare_dram_parameter("dbgQs", [N, GRP * N], bf16, isOutput=True)
        dbgRb = nc.declare_dram_parameter("dbgRb", [N, GRP * N], bf16, isOutput=True)

    with tile.TileContext(nc) as tc:
        with (
            tc.tile_pool(name="singles", bufs=1) as singles,
            tc.tile_pool(name="hpool", bufs=2) as hpool,
            tc.tile_pool(name="gpool", bufs=2) as gpool,
            tc.tile_pool(name="p1pool", bufs=1) as p1pool,
            tc.tile_pool(name="spool", bufs=2) as spool,
            tc.tile_pool(name="pa_h", bufs=2, space="PSUM") as pa_h,
            tc.tile_pool(name="pa_m", bufs=1, space="PSUM") as pa_m,
            tc.tile_pool(name="pa_t", bufs=1, space="PSUM") as pa_t,
            tc.tile_pool(name="pa_g", bufs=1, space="PSUM") as pa_g,
            tc.tile_pool(name="pa_p", bufs=1, space="PSUM") as pa_p,
            tc.tile_pool(name="pb", bufs=1, space="PSUM") as pb,
        ):
            csb = singles.tile([N, NCONST * GRP * N], f32)
            nc.sync.dma_start(out=csb, in_=constd[:, :])

            def cgrp(i):  # i-th [N, GRP, N] const group view
                return csb[:, i * GRP * N:(i + 1) * GRP * N].rearrange(
                    "p (g n) -> p g n", g=GRP)

            tauI4, I4, betaI4, m1I4 = cgrp(0), cgrp(1), cgrp(2), cgrp(3)
            ckI4 = [cgrp(4 + i) for i in range(DEG - 1)]  # k = DEG-2 .. 0

            Ibf = singles.tile([N, N], bf16)
            nc.vector.tensor_copy(Ibf, csb[:, GRP * N:GRP * N + N])  # I (bf16, exact)

            lacc = singles.tile([N, 8], f32)
            nc.vector.memset(lacc, 0.0)

            # FWL-padded split preconditioner (128 weight cols -> fast load)
            P1hi = singles.tile([N, HALF, 128], bf16)
            P1lo = singles.tile([N, HALF, 128], bf16)
            nc.vector.memset(P1hi, 0.0)
            nc.vector.memset(P1lo, 0.0)

            # per-iteration tanh bias columns (activation bias must be an AP)
            bias_n = singles.tile([N, L], f32)
            bias_p = singles.tile([N, L], f32)
            for ii in range(L):
                nc.gpsimd.memset(bias_n[:, ii:ii + 1], float(-1.0 / g3[ii]))
                nc.gpsimd.memset(bias_p[:, ii:ii + 1], float(1.0 / g3[ii]))

            for h in range(BS // HALF):
                hsl = slice(h * HALF, (h + 1) * HALF)
                HTyt = spool.tile([N, HALF], f32, tag="HTy")

                # ---------------- Phase A ----------------
                for c in range(HALF // CHUNK):
                    e0 = h * HALF + c * CHUNK
                    HF = hpool.tile([M, CHUNK, N + 1], f32, tag="HF")
                    nc.sync.dma_start(
                        out=HF[:, :, 0:N],
                        in_=Hd[e0:e0 + CHUNK].rearrange("e m n -> m e n"))
                    nc.sync.dma_start(out=HF[:, :, N], in_=yTd[:, e0:e0 + CHUNK])
                    Hhi = hpool.tile([M, CHUNK, N + 1], bf16, tag="Hhi")
                    nc.vector.tensor_copy(Hhi, HF)
                    Hlo = hpool.tile([M, CHUNK, N + 1], bf16, tag="Hlo")
                    nc.gpsimd.tensor_tensor(Hlo, HF, Hhi, AluOpType.subtract)

                    for g in range(CHUNK // GRP):
                        ph = pa_h.tile([N, GRP, N + 1], f32, tag="ph")
                        for j in range(GRP):
                            e = g * GRP + j
                            w_hi = Hhi[:, e, 0:N]
                            r_hi = Hhi[:, e, :]
                            r_lo = Hlo[:, e, :]
                            nc.tensor.matmul(ph[:, j, :], lhsT=w_hi, rhs=r_hi,
                                             start=True, stop=False)
                            nc.tensor.matmul(ph[:, j, :], lhsT=w_hi, rhs=r_lo,
                                             start=False, stop=False)
                            nc.tensor.matmul(ph[:, j, :], lhsT=Hlo[:, e, 0:N],
                                             rhs=r_hi, start=False, stop=True)
                        g0 = c * CHUNK + g * GRP  # index within half
                        # drains
                        nc.scalar.copy(HTyt[:, g0:g0 + GRP], ph[:, :, N])
                        Af = gpool.tile([N, GRP, N], f32, tag="Af")
                        nc.vector.tensor_tensor(Af, ph[:, :, 0:N], tauI4,
                                                AluOpType.add)
                        Ahi = gpool.tile([N, GRP, N], bf16, tag="Ahi")
                        nc.scalar.copy(Ahi, Af)
                        Alo = gpool.tile([N, GRP, N], bf16, tag="Alo")
                        nc.gpsimd.tensor_tensor(Alo, Af, Ahi, AluOpType.subtract)
                        Tb = gpool.tile([N, GRP, N], bf16, tag="Tb")
                        nc.vector.scalar_tensor_tensor(
                            Tb, Af, float(alpha), betaI4,
                            AluOpType.mult, AluOpType.add)
                        Mk = gpool.tile([N, GRP, N], bf16, tag="M0")
                        nc.vector.scalar_tensor_tensor(
                            Mk, Af, float(ct[DEG] * alpha), m1I4,
                            AluOpType.mult, AluOpType.add)
                        # Horner: M <- bf16(M @ Tb + ck I)
                        for i, k in enumerate(range(DEG - 2, -1, -1)):
                            pm = pa_m.tile([N, GRP, N], f32, tag="pm")
                            for j in range(GRP):
                                nc.tensor.matmul(pm[:, j, :], lhsT=Mk[:, j, :],
                                                 rhs=Tb[:, j, :],
                                                 start=True, stop=True)
                            Mn = gpool.tile([N, GRP, N], bf16, tag=f"M{1 + (i % 2)}")
                            nc.vector.tensor_tensor(Mn, pm, ckI4[i], AluOpType.add)
                            Mk = Mn
                        if DBG and h == 0 and c == 0 and g == 0:
                            nc.sync.dma_start(out=dbgAf[:, :], in_=Af.rearrange("p g n -> p (g n)"))
                            nc.sync.dma_start(out=dbgTb[:, :], in_=Tb.rearrange("p g n -> p (g n)"))
                            nc.sync.dma_start(out=dbgQ[:, :], in_=Mk.rearrange("p g n -> p (g n)"))
                        # symmetrize: Qs = bf16(Mk^T + Mk)
                        pt = pa_t.tile([N, GRP, N], bf16, tag="pt")
                        for j in range(GRP):
                            nc.tensor.transpose(pt[:, j, :], Mk[:, j, :], Ibf)
                        Qs = gpool.tile([N, GRP, N], bf16, tag="Qs")
                        nc.vector.tensor_tensor(Qs, pt, Mk, AluOpType.add)
                        # NS: G = A @ Qs ; Rb = bf16(I - G); P1 = Qs + Qs@Rb
                        if DBG and h == 0 and c == 0 and g == 0:
                            nc.sync.dma_start(out=dbgQs[:, :], in_=Qs.rearrange("p g n -> p (g n)"))
                        pg = pa_g.tile([N, GRP, N], f32, tag="pg")
                        for j in range(GRP):
                            nc.tensor.matmul(pg[:, j, :], lhsT=Ahi[:, j, :],
                                             rhs=Qs[:, j, :], start=True, stop=False)
                            nc.tensor.matmul(pg[:, j, :], lhsT=Alo[:, j, :],
                                             rhs=Qs[:, j, :], start=False, stop=True)
                        Rb = gpool.tile([N, GRP, N], bf16, tag="Rb")
                        nc.vector.tensor_tensor(Rb, I4, pg, AluOpType.subtract)
                        if DBG and h == 0 and c == 0 and g == 0:
                            nc.sync.dma_start(out=dbgRb[:, :], in_=Rb.rearrange("p g n -> p (g n)"))
                        pp = pa_p.tile([N, GRP, N], f32, tag="pp")
                        for j in range(GRP):
                            nc.tensor.matmul(pp[:, j, :], lhsT=Qs[:, j, :],
                                             rhs=Rb[:, j, :], start=True, stop=True)
                        P1f = gpool.tile([N, GRP, N], f32, tag="P1f")
                        nc.vector.tensor_tensor(P1f, pp, Qs, AluOpType.add)
                        nc.scalar.copy(P1hi[:, g0:g0 + GRP, 0:N], P1f)
                        nc.gpsimd.tensor_tensor(P1lo[:, g0:g0 + GRP, 0:N], P1f,
                                                P1hi[:, g0:g0 + GRP, 0:N],
                                                AluOpType.subtract)

                # ---------------- Phase B ----------------
                XI = spool.tile([N, HALF], f32, tag="XI")
                nc.sync.dma_start(out=XI, in_=xiTd[:, hsl])
                XT = spool.tile([N, HALF], f32, tag="XT")
                nc.sync.dma_start(out=XT, in_=xtTd[:, hsl])

                HB = HALF // 2
                u = [None, None]
                wpair = [None, None]
                xlo = [None, None]
                for blk in range(2):
                    bsl = slice(blk * HB, (blk + 1) * HB)
                    ub = spool.tile([N, HB], f32, tag=f"u{blk}")
                    nc.vector.memset(ub, 0.0)
                    u[blk] = ub
                    wb = spool.tile([N, HB], f32, tag=f"w{blk}")
                    nc.vector.scalar_tensor_tensor(
                        wb, XI[:, bsl], float(taus[0]), HTyt[:, bsl],
                        AluOpType.mult, AluOpType.add)
                    wp = spool.tile([N, 2 * HB], bf16, tag=f"wp{blk}")
                    nc.scalar.copy(wp[:, 0:2 * HB:2], wb)
                    nc.vector.tensor_tensor(wp[:, 1:2 * HB:2], wb,
                                            wp[:, 0:2 * HB:2], AluOpType.subtract)
                    wpair[blk] = wp

                for ii in range(L):
                    lo_iter = ii in (0, 2, 4, 6, 8, 9)
                    for blk in range(2):
                        bsl = slice(blk * HB, (blk + 1) * HB)
                        wp = wpair[blk]
                        px = pb.tile([128, 2 * HALF], f32, tag=f"px{blk}")
                        for eb in range(HB):
                            e = blk * HB + eb
                            nc.tensor.matmul(px[:, 4 * eb:4 * eb + 2],
                                             lhsT=P1hi[:, e, :],
                                             rhs=wp[:, 2 * eb:2 * eb + 2],
                                             start=True, stop=True)
                            if lo_iter:
                                nc.tensor.matmul(px[:, 4 * eb + 2:4 * eb + 4],
                                                 lhsT=P1lo[:, e, :],
                                                 rhs=wp[:, 2 * eb:2 * eb + 2],
                                                 start=True, stop=True)
                        if lo_iter:
                            xl = spool.tile([N, HB], f32, tag=f"xl{blk}")
                            nc.vector.tensor_reduce(
                                xl,
                                px[0:N, :].rearrange(
                                    "p (e four) -> p e four", four=4)[:, :, 2:4],
                                axis=mybir.AxisListType.X, op=AluOpType.add)
                            xlo[blk] = xl
                        xh = spool.tile([N, HB], f32, tag=f"xh{blk}")
                        nc.vector.tensor_reduce(
                            xh,
                            px[0:N, :].rearrange(
                                "p (e four) -> p e four", four=4)[:, :, 0:2],
                            axis=mybir.AxisListType.X, op=AluOpType.add)
                        x_sb = spool.tile([N, HB], f32, tag=f"x{blk}")
                        nc.vector.tensor_tensor(x_sb, xh, xlo[blk], AluOpType.add)
                        t = spool.tile([N, HB], f32, tag=f"t{blk}")
                        nc.vector.scalar_tensor_tensor(
                            t, u[blk], -float(ilbar[ii]), x_sb,
                            AluOpType.mult, AluOpType.add)
                        z1 = spool.tile([N, HB], f32, tag=f"z1{blk}")
                        nc.scalar.activation(z1, t, AF.Tanh,
                                             scale=float(0.5 / g1[ii]))
                        z2 = spool.tile([N, HB], f32, tag=f"z2{blk}")
                        nc.scalar.activation(z2, t, AF.Tanh,
                                             bias=bias_n[:, ii:ii + 1],
                                             scale=float(0.5 / g3[ii]))
                        z3 = spool.tile([N, HB], f32, tag=f"z3{blk}")
                        nc.scalar.activation(z3, t, AF.Tanh,
                                             bias=bias_p[:, ii:ii + 1],
                                             scale=float(0.5 / g3[ii]))
                        z12 = spool.tile([N, HB], f32, tag=f"z1{blk}")
                        nc.gpsimd.tensor_tensor(z12, z1, z2, AluOpType.add)
                        zn = spool.tile([N, HB], f32, tag=f"z{blk}")
                        nc.vector.tensor_tensor(zn, z12, z3, AluOpType.add)
                        huz = spool.tile([N, HB], f32, tag=f"huz{blk}")
                        nc.vector.scalar_tensor_tensor(
                            huz, zn, float(taus[ii]), u[blk],
                            AluOpType.mult, AluOpType.add)
                        un = spool.tile([N, HB], f32, tag=f"un{blk}")
                        nc.vector.scalar_tensor_tensor(
                            un, x_sb, -float(taus[ii]), huz,
                            AluOpType.mult, AluOpType.add)
                        u[blk] = un
                        if ii < L - 1:
                            hu = spool.tile([N, HB], f32, tag=f"hu{blk}")
                            nc.gpsimd.tensor_tensor(hu, HTyt[:, bsl], un,
                                                    AluOpType.add)
                            wn = spool.tile([N, HB], f32, tag=f"w{blk}")
                            nc.vector.scalar_tensor_tensor(
                                wn, zn, float(taus[ii + 1]), hu,
                                AluOpType.mult, AluOpType.add)
                            wp2 = spool.tile([N, 2 * HB], bf16, tag=f"wp{blk}")
                            nc.scalar.copy(wp2[:, 0:2 * HB:2], wn)
                            nc.vector.tensor_tensor(wp2[:, 1:2 * HB:2], wn,
                                                    wp2[:, 0:2 * HB:2],
                                                    AluOpType.subtract)
                            wpair[blk] = wp2
                        else:
                            dx = spool.tile([N, HB], f32, tag=f"t{blk}")
                            nc.vector.tensor_tensor(dx, x_sb, XT[:, bsl],
                                                    AluOpType.subtract)
                            dz = spool.tile([N, HB], f32, tag=f"z3{blk}")
                            nc.vector.tensor_tensor(dz, zn, XT[:, bsl],
                                                    AluOpType.subtract)
                            sq1 = spool.tile([N, HB], f32, tag=f"z2{blk}")
                            nc.scalar.activation(
                                sq1, dx, AF.Square,
                                accum_out=lacc[:, 4 * h + 2 * blk:4 * h + 2 * blk + 1])
                            sq2 = spool.tile([N, HB], f32, tag=f"huz{blk}")
                            nc.scalar.activation(
                                sq2, dz, AF.Square,
                                accum_out=lacc[:, 4 * h + 2 * blk + 1:4 * h + 2 * blk + 2])
                            nc.sync.dma_start(
                                out=xoutd[:, h * HALF + blk * HB:
                                          h * HALF + (blk + 1) * HB],
                                in_=x_sb)

            nc.sync.dma_start(out=loutd[:, :], in_=lacc)
    nc.compile()
    return nc


def _consts_array(tau0):
    ct = (_cheb_mono_t(DEG, tau0 + EV_LO, tau0 + EV_HI) * 0.5).astype(np.float64)
    a_, b_ = tau0 + EV_LO, tau0 + EV_HI
    beta = -2.0 * a_ / (b_ - a_) - 1.0
    I = np.eye(N, dtype=np.float32)
    tiles = [
        np.float32(tau0) * I,                               # tauI
        I,                                                  # I
        np.float32(beta) * I,                               # betaI
        np.float32(ct[DEG] * beta + ct[DEG - 1]) * I,       # m1I
    ]
    for k in range(DEG - 2, -1, -1):
        tiles.append(np.float32(ct[k]) * I)
    out = np.concatenate([np.tile(t, (1, GRP)) for t in tiles], axis=1)
    return np.ascontiguousarray(out, np.float32)


def _numpy_fallback(x_ini, y, H, xt, ILbar, tau, gamma1, gamma3):
    x = x_ini.copy(); z = x_ini.copy(); u = np.zeros_like(x_ini)
    Ht = np.swapaxes(H, 1, 2)
    HTH = Ht @ H
    HTy = Ht @ y
    eye = np.eye(H.shape[2], dtype=H.dtype)
    for ii in range(ILbar.shape[0]):
        A = HTH + tau[ii] * eye
        rhs = HTy + u + tau[ii] * z
        x = np.linalg.solve(A, rhs)
        t = x - u * ILbar[ii]
        z = (np.tanh(t * (0.5 / gamma1[ii]))
             + np.tanh((t - 2.0) * (0.5 / gamma3[ii]))
             + np.tanh((t + 2.0) * (0.5 / gamma3[ii])))
        u = u + tau[ii] * (z - x)
    dx = x - xt; dz = z - xt
    loss = (np.sum(dx * dx) + np.sum(dz * dz)).reshape(1, 1).astype(np.float32)
    return x.astype(np.float32), loss


def _ensure_ntff_hook():
    """The container's stub `antenv` lacks axon_hooks; bass_utils imports it
    when BASS_TRACE is set. Provide it (with the real ctypes-based NTFF hook
    when available) so tracing works and never crashes the run."""
    import sys, types
    if "antenv.axon_hooks" in sys.modules:
        return
    try:
        import antenv.axon_hooks  # noqa: F401
        return
    except Exception:
        pass
    mod = types.ModuleType("antenv.axon_hooks")
    mod._hook = None
    def set_axon_ntff_profile_hook(h):
        mod._hook = h
    def get_axon_ntff_profile_hook():
        return mod._hook
    mod.set_axon_ntff_profile_hook = set_axon_ntff_profile_hook
    mod.get_axon_ntff_profile_hook = get_axon_ntff_profile_hook
    sys.modules["antenv.axon_hooks"] = mod
    try:
        from trn_agent_boot.trn_boot import _ntff_profile_via_ctypes
        import os
        so = "/opt/axon/libaxon_pjrt.so"
        if os.path.exists(so):
            mod.set_axon_ntff_profile_hook(_ntff_profile_via_ctypes(so))
    except Exception:
        pass
    try:
        import concourse.bass_utils as _bu
        _orig = _bu.upload_artifacts
        def _safe_upload(d):
            try:
                return _orig(d)
            except Exception:
                return str(d)
        _bu.upload_artifacts = _safe_upload
    except Exception:
        pass


def kernel(**inputs):
    _ensure_ntff_hook()
    from concourse.bass_utils import run_bass_kernel_spmd

    H = np.ascontiguousarray(np.asarray(inputs["H"], np.float32))
    y = np.asarray(inputs["y"], np.float32)[:, :, 0]
    x_ini = np.asarray(inputs["x_ini"], np.float32)[:, :, 0]
    xt = np.asarray(inputs["xt"], np.float32)[:, :, 0]
    ILbar = np.asarray(inputs["ILbar"], np.float32)
    tau = np.asarray(inputs["tau"], np.float32)
    g1 = np.asarray(inputs["gamma1"], np.float32)
    g3 = np.asarray(inputs["gamma3"], np.float32)

    if not np.all(tau == tau[0]):
        return _numpy_fallback(
            np.asarray(inputs["x_ini"], np.float32), inputs["y"].astype(np.float32),
            H, np.asarray(inputs["xt"], np.float32), ILbar, tau, g1, g3)

    key = (float(tau[0]), tuple(ILbar.tolist()), tuple(tau.tolist()),
           tuple(g1.tolist()), tuple(g3.tolist()))
    if key not in _cache:
        _cache[key] = _build_graph(float(tau[0]), ILbar, tau, g1, g3)
    nc = _cache[key]

    consts = _consts_array(float(tau[0]))
    in_maps = []
    for c in range(NCORES):
        sl = slice(c * BS, (c + 1) * BS)
        in_maps.append({
            "H": H[sl],
            "yT": np.ascontiguousarray(y[sl].T),
            "xiT": np.ascontiguousarray(x_ini[sl].T),
            "xtT": np.ascontiguousarray(xt[sl].T),
            "consts": consts,
        })
    res = run_bass_kernel_spmd(nc, in_maps, core_ids=list(range(NCORES)))
    kernel.last_result = res

    xs = []
    loss = 0.0
    for c in range(NCORES):
        xs.append(np.asarray(res.results[c]["xout"]).T)
        loss += float(np.asarray(res.results[c]["lout"]).sum())
    x_full = np.concatenate(xs, axis=0)[:, :, None].astype(np.float32)
    return x_full, np.array([[loss]], dtype=np.float32)


# revision 39
# speedup vs baseline: 1.2204x; 1.2204x over previous
"""Trainium2 Bass kernel for the ADMM unrolled network (nn_ADMM).

Per batch element b (4096 total, data-parallel over 8 NeuronCores):
    A_b = H_b^T H_b + tau I   (80x80, SPD, tau constant across iterations)
    10 ADMM iterations, each requiring x = A_b^{-1} (HTy + u + tau z)
    plus elementwise tanh updates; final loss = sum |x-xt|^2 + |z-xt|^2.

Algorithm on device (per element):
  Phase A (TensorE-heavy):
    - split-bf16 exact HTH+HTy (Hhi/Hlo bf16 pair, 3 matmuls, fp32 PSUM)
    - Q = Chebyshev poly p_d(A)/2 approx of inv(A)/2, Horner in the
      normalized variable T = alpha A + beta I, bf16 intermediates
    - Qs = Q + Q^T (PE transpose; exactly symmetric)
    - one non-commutative Newton-Schulz step vs split-A:
        P1 = Qs + Qs (I - A Qs)   (valid: residual squares exactly)
      P1 stored fp32.  Applied operator is P1^T; ||I - P1^T A|| ~ 4e-5.
      P1 split into an exact bf16 pair (P1hi + P1lo), FWL-padded to 128
      weight columns for fast weight loads.
  Phase B: 10 iterations; per element x = P1^T w via two N=2 bf16 matmuls
    (stationary P1hi / P1lo, moving [w_hi | w_lo] split columns); the P1lo
    term is refreshed on iterations {0,2,4,6,8,9} and reused stale on the
    others.  Elementwise phase runs feature-major [80, 128] in two
    independently pipelined blocks on DVE/ACT/GPSIMD (tanh via ScalarE LUT).
  Measured: HW exec ~1.06 ms (fast clock mode) / ~1.27 ms (P0 downclock),
    rel_err(x) ~2.4e-3, rel_err(loss) ~1.1e-5.

Everything (shapes, tau, gammas, Chebyshev coefficients) is resolved at
kernel() call time from the actual inputs and baked into the graph.
"""

import numpy as np

B, M, N, L = 4096, 100, 80, 10
NCORES = 8
BS = B // NCORES          # 512 elements per core
HALF = 256                # phase-B batch (2 per core)
CHUNK = 16                # H DMA chunk
GRP = 4                   # phase-A group (psum batching)
DEG = 3                   # Chebyshev degree
DBG = False               # debug output dumps (small sizes only)
EV_LO, EV_HI = 0.0, 410.0  # eigenvalue range of HTH (empirical + margin)

_cache = {}


def _cheb_mono_t(deg, a, b):
    """Monomial coeffs (in t) of near-minimax poly approx of 1/x on [a,b],
    x = ((t+1)/2)*(b-a) + a."""
    kk = np.arange(deg + 1)
    xs = 0.5 * (b - a) * np.cos((2 * kk + 1) * np.pi / (2 * (deg + 1))) + 0.5 * (a + b)
    cc = np.polynomial.chebyshev.chebfit(2 * (xs - a) / (b - a) - 1, 1.0 / xs, deg)
    return np.polynomial.chebyshev.cheb2poly(cc)


def _build_graph(tau0, ilbar, taus, g1, g3):
    import concourse.bass as bass
    import concourse.mybir as mybir
    import concourse.tile as tile
    from concourse import bacc
    from concourse.alu_op_type import AluOpType

    f32 = mybir.dt.float32
    bf16 = mybir.dt.bfloat16
    AF = mybir.ActivationFunctionType

    a_, b_ = tau0 + EV_LO, tau0 + EV_HI
    alpha = 2.0 / (b_ - a_)
    beta = -2.0 * a_ / (b_ - a_) - 1.0
    ct = (_cheb_mono_t(DEG, a_, b_) * 0.5).astype(np.float32)  # halved (symmetrize)

    nc = bacc.Bacc("TRN2")
    Hd = nc.declare_dram_parameter("H", [BS, M, N], f32, isOutput=False)
    yTd = nc.declare_dram_parameter("yT", [M, BS], f32, isOutput=False)
    xiTd = nc.declare_dram_parameter("xiT", [N, BS], f32, isOutput=False)
    xtTd = nc.declare_dram_parameter("xtT", [N, BS], f32, isOutput=False)
    # consts layout (fp32): tauI | I | betaI4 | m1I4 | ckI4[k=DEG-2..0]
    CW = 2 * N + (2 + (DEG - 1)) * GRP * N
    constd = nc.declare_dram_parameter("consts", [N, CW], f32, isOutput=False)
    xoutd = nc.declare_dram_parameter("xout", [N, BS], f32, isOutput=True)
    loutd = nc.declare_dram_parameter("lout", [N, 8], f32, isOutput=True)
    if DBG:
        dbgP1h = nc.declare_dram_parameter("dbgP1h", [N, HALF * N], bf16, isOutput=True)
        dbgP1l = nc.declare_dram_parameter("dbgP1l", [N, HALF * N], bf16, isOutput=True)
        dbgHTy = nc.declare_dram_parameter("dbgHTy", [N, HALF], f32, isOutput=True)
        dbgW = nc.declare_dram_parameter("dbgW", [N, HALF], f32, isOutput=True)
        dbgX0 = nc.declare_dram_parameter("dbgX0", [N, HALF], f32, isOutput=True)
        dbgAf = nc.declare_dram_parameter("dbgAf", [N, GRP * N], f32, isOutput=True)
        dbgTb = nc.declare_dram_parameter("dbgTb", [N, GRP * N], bf16, isOutput=True)
        dbgQ = nc.declare_dram_parameter("dbgQ", [N, GRP * N], bf16, isOutput=True)
        dbgQs = nc.declare_dram_parameter("dbgQs", [N, GRP * N], bf16, isOutput=True)
        dbgRb = nc.declare_dram_parameter("dbgRb", [N, GRP * N], bf16, isOutput=True)

    with tile.TileContext(nc) as tc:
        with (
            tc.tile_pool(name="singles", bufs=1) as singles,
            tc.tile_pool(name="hpool", bufs=2) as hpool,
            tc.tile_pool(name="gpool", bufs=3) as gpool,
            tc.tile_pool(name="p1pool", bufs=1) as p1pool,
            tc.tile_pool(name="spool", bufs=2) as spool,
            tc.tile_pool(name="ropool", bufs=1) as ropool,
            tc.tile_pool(name="pa_h", bufs=2, space="PSUM") as pa_h,
            tc.tile_pool(name="pa_m", bufs=1, space="PSUM") as pa_m,
            tc.tile_pool(name="pa_t", bufs=1, space="PSUM") as pa_t,
            tc.tile_pool(name="pa_g", bufs=1, space="PSUM") as pa_g,
            tc.tile_pool(name="pa_p", bufs=1, space="PSUM") as pa_p,
            tc.tile_pool(name="pb", bufs=1, space="PSUM") as pb,
        ):
            csb = singles.tile([N, CW], f32)
            nc.sync.dma_start(out=csb, in_=constd[:, :])

            def cgrp(i):  # i-th [N, GRP, N] const group view (after 2N singles)
                return csb[:, 2 * N + i * GRP * N:2 * N + (i + 1) * GRP * N].rearrange(
                    "p (g n) -> p g n", g=GRP)

            def cbro(off):  # broadcast one [N, N] const across the group dim
                import concourse.bass as _b
                base = csb[:, off:off + N]
                return _b.AP(tensor=base.tensor, offset=base.offset,
                             ap=[base.ap[0], [0, GRP], base.ap[1]])

            tauI4, I4 = cbro(0), cbro(N)
            betaI4, m1I4 = cgrp(0), cgrp(1)
            ckI4 = [cgrp(2 + i) for i in range(DEG - 1)]  # k = DEG-2 .. 0

            Ibf = singles.tile([N, N], bf16)
            nc.vector.tensor_copy(Ibf, csb[:, N:2 * N])  # I (bf16, exact)

            lacc = singles.tile([N, 8], f32)
            nc.vector.memset(lacc, 0.0)

            # FWL-padded split preconditioner (128 weight cols -> fast load)
            P1hi = singles.tile([N, HALF, 128], bf16)
            P1lo = singles.tile([N, HALF, 128], bf16)
            nc.vector.memset(P1hi, 0.0)
            nc.vector.memset(P1lo, 0.0)

            # per-iteration tanh bias columns (activation bias must be an AP)
            bias_n = singles.tile([N, L], f32)
            bias_p = singles.tile([N, L], f32)
            for ii in range(L):
                nc.gpsimd.memset(bias_n[:, ii:ii + 1], float(-1.0 / g3[ii]))
                nc.gpsimd.memset(bias_p[:, ii:ii + 1], float(1.0 / g3[ii]))

            for h in range(BS // HALF):
                hsl = slice(h * HALF, (h + 1) * HALF)
                HTyt = spool.tile([N, HALF], f32, tag="HTy")

                # ---------------- Phase A ----------------
                for c in range(HALF // CHUNK):
                    e0 = h * HALF + c * CHUNK
                    HF = hpool.tile([M, CHUNK, N + 1], f32, tag="HF")
                    nc.sync.dma_start(
                        out=HF[:, :, 0:N],
                        in_=Hd[e0:e0 + CHUNK].rearrange("e m n -> m e n"))
                    nc.sync.dma_start(out=HF[:, :, N], in_=yTd[:, e0:e0 + CHUNK])
                    Hhi = hpool.tile([M, CHUNK, N + 1], bf16, tag="Hhi")
                    nc.vector.tensor_copy(Hhi, HF)
                    Hlo = hpool.tile([M, CHUNK, N + 1], bf16, tag="Hlo")
                    nc.gpsimd.tensor_tensor(Hlo, HF, Hhi, AluOpType.subtract)

                    for g in range(CHUNK // GRP):
                        ph = pa_h.tile([N, GRP, N + 1], f32, tag="ph")
                        for j in range(GRP):
                            e = g * GRP + j
                            w_hi = Hhi[:, e, 0:N]
                            r_hi = Hhi[:, e, :]
                            r_lo = Hlo[:, e, :]
                            nc.tensor.matmul(ph[:, j, :], lhsT=w_hi, rhs=r_hi,
                                             start=True, stop=False)
                            nc.tensor.matmul(ph[:, j, :], lhsT=w_hi, rhs=r_lo,
                                             start=False, stop=False)
                            nc.tensor.matmul(ph[:, j, :], lhsT=Hlo[:, e, 0:N],
                                             rhs=r_hi, start=False, stop=True)
                        g0 = c * CHUNK + g * GRP  # index within half
                        # drains
                        nc.scalar.copy(HTyt[:, g0:g0 + GRP], ph[:, :, N])
                        Af = gpool.tile([N, GRP, N], f32, tag="Af")
                        nc.vector.tensor_tensor(Af, ph[:, :, 0:N], tauI4,
                                                AluOpType.add)
                        Ahi = gpool.tile([N, GRP, N], bf16, tag="Ahi")
                        nc.scalar.copy(Ahi, Af)
                        Alo = gpool.tile([N, GRP, N], bf16, tag="Alo")
                        nc.gpsimd.tensor_tensor(Alo, Af, Ahi, AluOpType.subtract)
                        Tb = gpool.tile([N, GRP, N], bf16, tag="Tb")
                        nc.vector.scalar_tensor_tensor(
                            Tb, Af, float(alpha), betaI4,
                            AluOpType.mult, AluOpType.add)
                        Mk = gpool.tile([N, GRP, N], bf16, tag="M0")
                        nc.vector.scalar_tensor_tensor(
                            Mk, Af, float(ct[DEG] * alpha), m1I4,
                            AluOpType.mult, AluOpType.add)
                        # Horner: M <- bf16(M @ Tb + ck I)
                        for i, k in enumerate(range(DEG - 2, -1, -1)):
                            pm = pa_m.tile([N, GRP, N], f32, tag="pm")
                            for j in range(GRP):
                                nc.tensor.matmul(pm[:, j, :], lhsT=Mk[:, j, :],
                                                 rhs=Tb[:, j, :],
                                                 start=True, stop=True)
                            Mn = gpool.tile([N, GRP, N], bf16, tag=f"M{1 + (i % 2)}")
                            nc.vector.tensor_tensor(Mn, pm, ckI4[i], AluOpType.add)
                            Mk = Mn
                        if DBG and h == 0 and c == 0 and g == 0:
                            nc.sync.dma_start(out=dbgAf[:, :], in_=Af.rearrange("p g n -> p (g n)"))
                            nc.sync.dma_start(out=dbgTb[:, :], in_=Tb.rearrange("p g n -> p (g n)"))
                            nc.sync.dma_start(out=dbgQ[:, :], in_=Mk.rearrange("p g n -> p (g n)"))
                        # symmetrize: Qs = bf16(Mk^T + Mk)
                        pt = pa_t.tile([N, GRP, N], bf16, tag="pt")
                        for j in range(GRP):
                            nc.tensor.transpose(pt[:, j, :], Mk[:, j, :], Ibf)
                        Qs = gpool.tile([N, GRP, N], bf16, tag="Qs")
                        nc.vector.tensor_tensor(Qs, pt, Mk, AluOpType.add)
                        # NS: G = A @ Qs ; Rb = bf16(I - G); P1 = Qs + Qs@Rb
                        if DBG and h == 0 and c == 0 and g == 0:
                            nc.sync.dma_start(out=dbgQs[:, :], in_=Qs.rearrange("p g n -> p (g n)"))
                        pg = pa_g.tile([N, GRP, N], f32, tag="pg")
                        for j in range(GRP):
                            nc.tensor.matmul(pg[:, j, :], lhsT=Ahi[:, j, :],
                                             rhs=Qs[:, j, :], start=True, stop=False)
                            nc.tensor.matmul(pg[:, j, :], lhsT=Alo[:, j, :],
                                             rhs=Qs[:, j, :], start=False, stop=True)
                        Rb = gpool.tile([N, GRP, N], bf16, tag="Rb")
                        nc.vector.tensor_tensor(Rb, I4, pg, AluOpType.subtract)
                        if DBG and h == 0 and c == 0 and g == 0:
                            nc.sync.dma_start(out=dbgRb[:, :], in_=Rb.rearrange("p g n -> p (g n)"))
                        pp = pa_p.tile([N, GRP, N], f32, tag="pp")
                        for j in range(GRP):
                            nc.tensor.matmul(pp[:, j, :], lhsT=Qs[:, j, :],
                                             rhs=Rb[:, j, :], start=True, stop=True)
                        P1f = gpool.tile([N, GRP, N], f32, tag="P1f")
                        nc.vector.tensor_tensor(P1f, pp, Qs, AluOpType.add)
                        nc.scalar.copy(P1hi[:, g0:g0 + GRP, 0:N], P1f)
                        nc.gpsimd.tensor_tensor(P1lo[:, g0:g0 + GRP, 0:N], P1f,
                                                P1hi[:, g0:g0 + GRP, 0:N],
                                                AluOpType.subtract)

                # ---------------- Phase B ----------------
                XI = ropool.tile([N, HALF], f32, tag="XI")
                nc.sync.dma_start(out=XI, in_=xiTd[:, hsl])
                XT = ropool.tile([N, HALF], f32, tag="XT")
                nc.sync.dma_start(out=XT, in_=xtTd[:, hsl])

                HB = HALF // 2
                u = [None, None]
                wpair = [None, None]
                xlo = [None, None]
                for blk in range(2):
                    bsl = slice(blk * HB, (blk + 1) * HB)
                    ub = spool.tile([N, HB], f32, tag=f"u{blk}")
                    nc.vector.memset(ub, 0.0)
                    u[blk] = ub
                    wb = spool.tile([N, HB], f32, tag=f"w{blk}")
                    nc.vector.scalar_tensor_tensor(
                        wb, XI[:, bsl], float(taus[0]), HTyt[:, bsl],
                        AluOpType.mult, AluOpType.add)
                    wp = spool.tile([N, 2 * HB], bf16, tag=f"wp{blk}")
                    nc.scalar.copy(wp[:, 0:2 * HB:2], wb)
                    nc.vector.tensor_tensor(wp[:, 1:2 * HB:2], wb,
                                            wp[:, 0:2 * HB:2], AluOpType.subtract)
                    wpair[blk] = wp

                for ii in range(L):
                    lo_iter = ii in (0, 2, 4, 6, 8, 9)
                    for blk in range(2):
                        bsl = slice(blk * HB, (blk + 1) * HB)
                        wp = wpair[blk]
                        px = pb.tile([128, 2 * HALF], f32, tag=f"px{blk}")
                        for eb in range(HB):
                            e = blk * HB + eb
                            nc.tensor.matmul(px[:, 4 * eb:4 * eb + 2],
                                             lhsT=P1hi[:, e, :],
                                             rhs=wp[:, 2 * eb:2 * eb + 2],
                                             start=True, stop=True)
                            if lo_iter:
                                nc.tensor.matmul(px[:, 4 * eb + 2:4 * eb + 4],
                                                 lhsT=P1lo[:, e, :],
                                                 rhs=wp[:, 2 * eb:2 * eb + 2],
                                                 start=True, stop=True)
                        if lo_iter:
                            xl = spool.tile([N, HB], f32, tag=f"xl{blk}")
                            nc.vector.tensor_reduce(
                                xl,
                                px[0:N, :].rearrange(
                                    "p (e four) -> p e four", four=4)[:, :, 2:4],
                                axis=mybir.AxisListType.X, op=AluOpType.add)
                            xlo[blk] = xl
                        xh = spool.tile([N, HB], f32, tag=f"huz{blk}")
                        nc.vector.tensor_reduce(
                            xh,
                            px[0:N, :].rearrange(
                                "p (e four) -> p e four", four=4)[:, :, 0:2],
                            axis=mybir.AxisListType.X, op=AluOpType.add)
                        x_sb = spool.tile([N, HB], f32, tag=f"x{blk}")
                        nc.vector.tensor_tensor(x_sb, xh, xlo[blk], AluOpType.add)
                        t = spool.tile([N, HB], f32, tag=f"t{blk}")
                        nc.vector.scalar_tensor_tensor(
                            t, u[blk], -float(ilbar[ii]), x_sb,
                            AluOpType.mult, AluOpType.add)
                        z1 = spool.tile([N, HB], f32, tag=f"z1{blk}")
                        nc.scalar.activation(z1, t, AF.Tanh,
                                             scale=float(0.5 / g1[ii]))
                        z2 = spool.tile([N, HB], f32, tag=f"z2{blk}")
                        nc.scalar.activation(z2, t, AF.Tanh,
                                             bias=bias_n[:, ii:ii + 1],
                                             scale=float(0.5 / g3[ii]))
                        z3 = spool.tile([N, HB], f32, tag=f"z3{blk}")
                        nc.scalar.activation(z3, t, AF.Tanh,
                                             bias=bias_p[:, ii:ii + 1],
                                             scale=float(0.5 / g3[ii]))
                        z12 = spool.tile([N, HB], f32, tag=f"z1{blk}")
                        nc.gpsimd.tensor_tensor(z12, z1, z2, AluOpType.add)
                        zn = spool.tile([N, HB], f32, tag=f"z{blk}")
                        nc.vector.tensor_tensor(zn, z12, z3, AluOpType.add)
                        huz = spool.tile([N, HB], f32, tag=f"huz{blk}")
                        nc.vector.scalar_tensor_tensor(
                            huz, zn, float(taus[ii]), u[blk],
                            AluOpType.mult, AluOpType.add)
                        un = spool.tile([N, HB], f32, tag=f"un{blk}")
                        nc.vector.scalar_tensor_tensor(
                            un, x_sb, -float(taus[ii]), huz,
                            AluOpType.mult, AluOpType.add)
                        u[blk] = un
                        if ii < L - 1:
                            hu = spool.tile([N, HB], f32, tag=f"hu{blk}")
                            nc.gpsimd.tensor_tensor(hu, HTyt[:, bsl], un,
                                                    AluOpType.add)
                            wn = spool.tile([N, HB], f32, tag=f"w{blk}")
                            nc.vector.scalar_tensor_tensor(
                                wn, zn, float(taus[ii + 1]), hu,
                                AluOpType.mult, AluOpType.add)
                            wp2 = spool.tile([N, 2 * HB], bf16, tag=f"wp{blk}")
                            nc.scalar.copy(wp2[:, 0:2 * HB:2], wn)
                            nc.vector.tensor_tensor(wp2[:, 1:2 * HB:2], wn,
                                                    wp2[:, 0:2 * HB:2],
                                                    AluOpType.subtract)
                            wpair[blk] = wp2
                        else:
                            dx = spool.tile([N, HB], f32, tag=f"t{blk}")
                            nc.vector.tensor_tensor(dx, x_sb, XT[:, bsl],
                                                    AluOpType.subtract)
                            dz = spool.tile([N, HB], f32, tag=f"z3{blk}")
                            nc.vector.tensor_tensor(dz, zn, XT[:, bsl],
                                                    AluOpType.subtract)
                            sq1 = spool.tile([N, HB], f32, tag=f"z2{blk}")
                            nc.scalar.activation(
                                sq1, dx, AF.Square,
                                accum_out=lacc[:, 4 * h + 2 * blk:4 * h + 2 * blk + 1])
                            sq2 = spool.tile([N, HB], f32, tag=f"huz{blk}")
                            nc.scalar.activation(
                                sq2, dz, AF.Square,
                                accum_out=lacc[:, 4 * h + 2 * blk + 1:4 * h + 2 * blk + 2])
                            nc.sync.dma_start(
                                out=xoutd[:, h * HALF + blk * HB:
                                          h * HALF + (blk + 1) * HB],
                                in_=x_sb)

            nc.sync.dma_start(out=loutd[:, :], in_=lacc)
    nc.compile()
    return nc


def _consts_array(tau0):
    ct = (_cheb_mono_t(DEG, tau0 + EV_LO, tau0 + EV_HI) * 0.5).astype(np.float64)
    a_, b_ = tau0 + EV_LO, tau0 + EV_HI
    beta = -2.0 * a_ / (b_ - a_) - 1.0
    I = np.eye(N, dtype=np.float32)
    singles = [np.float32(tau0) * I, I]
    grouped = [
        np.float32(beta) * I,                               # betaI
        np.float32(ct[DEG] * beta + ct[DEG - 1]) * I,       # m1I
    ]
    for k in range(DEG - 2, -1, -1):
        grouped.append(np.float32(ct[k]) * I)
    out = np.concatenate(singles + [np.tile(t, (1, GRP)) for t in grouped], axis=1)
    return np.ascontiguousarray(out, np.float32)


def _numpy_fallback(x_ini, y, H, xt, ILbar, tau, gamma1, gamma3):
    x = x_ini.copy(); z = x_ini.copy(); u = np.zeros_like(x_ini)
    Ht = np.swapaxes(H, 1, 2)
    HTH = Ht @ H
    HTy = Ht @ y
    eye = np.eye(H.shape[2], dtype=H.dtype)
    for ii in range(ILbar.shape[0]):
        A = HTH + tau[ii] * eye
        rhs = HTy + u + tau[ii] * z
        x = np.linalg.solve(A, rhs)
        t = x - u * ILbar[ii]
        z = (np.tanh(t * (0.5 / gamma1[ii]))
             + np.tanh((t - 2.0) * (0.5 / gamma3[ii]))
             + np.tanh((t + 2.0) * (0.5 / gamma3[ii])))
        u = u + tau[ii] * (z - x)
    dx = x - xt; dz = z - xt
    loss = (np.sum(dx * dx) + np.sum(dz * dz)).reshape(1, 1).astype(np.float32)
    return x.astype(np.float32), loss


def _ensure_ntff_hook():
    """The container's stub `antenv` lacks axon_hooks; bass_utils imports it
    when BASS_TRACE is set. Provide it (with the real ctypes-based NTFF hook
    when available) so tracing works and never crashes the run."""
    import sys, types
    if "antenv.axon_hooks" in sys.modules:
        return
    try:
        import antenv.axon_hooks  # noqa: F401
        return
    except Exception:
        pass
    mod = types.ModuleType("antenv.axon_hooks")
    mod._hook = None
    def set_axon_ntff_profile_hook(h):
        mod._hook = h
    def get_axon_ntff_profile_hook():
        return mod._hook
    mod.set_axon_ntff_profile_hook = set_axon_ntff_profile_hook
    mod.get_axon_ntff_profile_hook = get_axon_ntff_profile_hook
    sys.modules["antenv.axon_hooks"] = mod
    try:
        from trn_agent_boot.trn_boot import _ntff_profile_via_ctypes
        import os
        so = "/opt/axon/libaxon_pjrt.so"
        if os.path.exists(so):
            mod.set_axon_ntff_profile_hook(_ntff_profile_via_ctypes(so))
    except Exception:
        pass
    try:
        import concourse.bass_utils as _bu
        _orig = _bu.upload_artifacts
        def _safe_upload(d):
            try:
                return _orig(d)
            except Exception:
                return str(d)
        _bu.upload_artifacts = _safe_upload
    except Exception:
        pass


def kernel(**inputs):
    _ensure_ntff_hook()
    from concourse.bass_utils import run_bass_kernel_spmd

    H = np.ascontiguousarray(np.asarray(inputs["H"], np.float32))
    y = np.asarray(inputs["y"], np.float32)[:, :, 0]
    x_ini = np.asarray(inputs["x_ini"], np.float32)[:, :, 0]
    xt = np.asarray(inputs["xt"], np.float32)[:, :, 0]
    ILbar = np.asarray(inputs["ILbar"], np.float32)
    tau = np.asarray(inputs["tau"], np.float32)
    g1 = np.asarray(inputs["gamma1"], np.float32)
    g3 = np.asarray(inputs["gamma3"], np.float32)

    if not np.all(tau == tau[0]):
        return _numpy_fallback(
            np.asarray(inputs["x_ini"], np.float32), inputs["y"].astype(np.float32),
            H, np.asarray(inputs["xt"], np.float32), ILbar, tau, g1, g3)

    key = (float(tau[0]), tuple(ILbar.tolist()), tuple(tau.tolist()),
           tuple(g1.tolist()), tuple(g3.tolist()))
    if key not in _cache:
        _cache[key] = _build_graph(float(tau[0]), ILbar, tau, g1, g3)
    nc = _cache[key]

    consts = _consts_array(float(tau[0]))
    in_maps = []
    for c in range(NCORES):
        sl = slice(c * BS, (c + 1) * BS)
        in_maps.append({
            "H": H[sl],
            "yT": np.ascontiguousarray(y[sl].T),
            "xiT": np.ascontiguousarray(x_ini[sl].T),
            "xtT": np.ascontiguousarray(xt[sl].T),
            "consts": consts,
        })
    res = run_bass_kernel_spmd(nc, in_maps, core_ids=list(range(NCORES)))
    kernel.last_result = res

    xs = []
    loss = 0.0
    for c in range(NCORES):
        xs.append(np.asarray(res.results[c]["xout"]).T)
        loss += float(np.asarray(res.results[c]["lout"]).sum())
    x_full = np.concatenate(xs, axis=0)[:, :, None].astype(np.float32)
    return x_full, np.array([[loss]], dtype=np.float32)
